# revision 22
# baseline (speedup 1.0000x reference)
"""Trainium2 Bass kernel for nn_AvgPoolVectorsPerWSI (segment-mean over groups).

Math: x [N=2048, M=512, 7, 7], idx [N] in [0,64)
  out[g, m] = mean over {n: idx[n]==g} and spatial of x[n, m, :, :]  -> [64, 512, 1, 1]

Strategy (no collectives needed):
  - Shard over M: core k handles an m-slice of 64 channels, reading its x
    slice [2048, 64, 49] (25.7 MB) exactly once.  The 16 SDMA engines
    stream it at ~427 GB/s; 16 uniform 128-row rounds on two HWDGE queues
    (SP = even rounds + tail pieces + out, ACT = odd rounds).
  - aux (iota/scale/idx) is the FIRST transfer on the SP ring so w
    generation starts at ~9 us instead of ~20 us.
  - Column layout per row is [MV=44 chans for VectorE | MC=20 chans for
    TensorE].  DVE j-reduces 44 chans (~2.4 us/round), PE segment-sums 20
    chans via two psum-bank-aligned chunk matmuls (~2.3 us/round), so the
    PE chain after the last landing is two matmuls, not three.  (A HW
    duty-cycle limiter - ham k=4/8, 3413 ns quanta - randomly halves SDMA
    engine 15's rate on ~half of runs regardless of schedule; compute
    starting too early, e.g. aux as the first transfer, seemed to raise
    that incidence, so aux stays second.)
  - Per-transfer/per-slot semaphores only (each threshold 16*generation
    gates the LATEST transfer on that sem) - sound even when one SDMA
    engine lags several transfers behind the other 15, which happens
    under throttling.
  - The ACT queue gets only 6 rounds (1..11) so it empties ~54 us in;
    the SP queue alone then delivers rounds 12..15 serially at the full
    ~427 GB/s (one round per 3.76 us) instead of the two queues bunching
    the last rounds together faster than DVE's 2.2 us/round can drain.
    Rounds 14/15 land as column pieces (r14: MV|PE, r15: three MV thirds
    then the two PE chunks last - the PE post-chain is shortest), and
    round 15 runs [chunk0, small, chunk1] on the PE so the psum_small ->
    copy -> out path starts early.  ACT does the psum_small -> SBUF copy
    (gated on its own completion sem before the DMA reads it) and issues
    the big output piece itself, in parallel with the SP queue's two
    SUBRED-gated pieces.
  - All compute is fp32-exact.  Host concatenates the 8 per-core results
    along m.

Raw Block implementation (not Tile): the walrus matmul/DMA lowerings only
accept ONE attached sync-wait per instruction; standalone wait_ge
instructions sidestep that.
"""

from contextlib import ExitStack

import numpy as np

import concourse.bass as bass
import concourse.mybir as mybir
from concourse.bass_utils import run_bass_kernel_spmd

N = 2048
M = 512
HW = 49
G = 64
CORES = 8
ML = M // CORES       # 64
F = ML * HW           # 3136
P = 128
NT = N // P           # 16
BUFS = 8

MC = 20               # TensorE channels
MV = ML - MC          # 44 VectorE channels
DOFF = MV * HW        # 2156: PE region starts here (MV region first)
FC = MC * HW          # 980
CHUNKS = [(0, 512), (512, FC)]                 # psum-bank-aligned chunks
NCH = len(CHUNKS)
# epilogue j-reduce pieces of psum_big: (m range, pe_big threshold)
SUBRED = [
    ((0, 10), (NT - 1) * NCH + 1),
    ((10, MC), NT * NCH),
]

F32 = mybir.dt.float32

PEN = NT - 2          # round 14: [MV | PE] pieces
LAST = NT - 1         # round 15: MV thirds, then PE chunks last
MV3A = 15 * HW        # 735
MV3B = 30 * HW        # 1470
R15 = [(0, MV3A), (MV3A, MV3B), (MV3B, DOFF),
       (DOFF + CHUNKS[0][0], DOFF + CHUNKS[0][1]),
       (DOFF + CHUNKS[1][0], DOFF + CHUNKS[1][1])]


def _cum(r):
    """Threshold when slot-sem (slot r % BUFS) shows round r landed."""
    return 16 * (r // BUFS + 1)


def _build():
    nc = bass.Bass(trn_type="TRN2", target_bir_lowering=False)
    x_ext = nc.declare_dram_parameter("x", [N, F], F32, isOutput=False)
    aux_ext = nc.declare_dram_parameter("aux", [P, G + G + NT], F32,
                                        isOutput=False)
    out_ext = nc.declare_dram_parameter("out", [G, ML], F32, isOutput=True)

    x_t = x_ext.ap().rearrange("(t p) f -> t p f", p=P)

    with ExitStack() as ctx:
        x_buf = ctx.enter_context(nc.sbuf_tensor([P, BUFS * F], F32))
        xs_buf = ctx.enter_context(nc.sbuf_tensor([P, BUFS * MV], F32))
        aux_sb = ctx.enter_context(nc.sbuf_tensor([P, G + G + NT], F32))
        w_sb = ctx.enter_context(nc.sbuf_tensor([P, NT * G], F32))
        out_sb = ctx.enter_context(nc.sbuf_tensor([G, ML], F32))
        psum_big = ctx.enter_context(nc.psum_tensor([G, FC], F32))
        psum_small = ctx.enter_context(nc.psum_tensor([G, MV], F32))
        dma_x = [
            ctx.enter_context(nc.semaphore(name=f"dma_x{s}"))
            for s in range(BUFS)
        ]
        dma_a = ctx.enter_context(nc.semaphore(name="dma_a"))
        dma_q = [
            ctx.enter_context(nc.semaphore(name=f"dma_q{i}"))
            for i in range(2)
        ]  # round 14's MV / PE piece
        dma_p = [
            ctx.enter_context(nc.semaphore(name=f"dma_p{i}"))
            for i in range(len(R15))
        ]  # round 15's pieces
        dma_o = ctx.enter_context(nc.semaphore(name="dma_o"))
        wg_sem = ctx.enter_context(nc.semaphore(name="wg"))
        red_sem = ctx.enter_context(nc.semaphore(name="red"))
        pe_big = ctx.enter_context(nc.semaphore(name="pe_big"))
        pe_tile = ctx.enter_context(nc.semaphore(name="pe_tile"))
        fin_sem = ctx.enter_context(nc.semaphore(name="fin"))
        cp_sem = ctx.enter_context(nc.semaphore(name="cp"))
        block = ctx.enter_context(nc.Block())

        # ---- DMA program A (SP / HWDGE): aux, even rounds, tail, out ----
        @block.sync
        def _(sync):
            for r in list(range(0, NT - 4, 2)) + [NT - 4, NT - 3]:
                if r >= BUFS:
                    sync.wait_ge(pe_tile, r - BUFS + 1)
                slot = r % BUFS
                sync.dma_start(
                    out=x_buf[:, slot * F:(slot + 1) * F], in_=x_t[r]
                ).then_inc(dma_x[slot], 16)
                if r == 0:
                    sync.dma_start(
                        out=aux_sb[:, :], in_=aux_ext.ap()
                    ).then_inc(dma_a, 16)
            # round 14 as MV | PE pieces
            qslot = PEN % BUFS
            sync.wait_ge(pe_tile, PEN - BUFS + 1)
            sync.dma_start(
                out=x_buf[:, qslot * F:qslot * F + DOFF],
                in_=x_t[PEN][:, 0:DOFF],
            ).then_inc(dma_q[0], 16)
            sync.dma_start(
                out=x_buf[:, qslot * F + DOFF:(qslot + 1) * F],
                in_=x_t[PEN][:, DOFF:F],
            ).then_inc(dma_q[1], 16)
            # round 15: PE piece first, then three MV thirds
            slot = LAST % BUFS
            sync.wait_ge(pe_tile, LAST - BUFS + 1)
            for i, (lo, hi) in enumerate(R15):
                sync.dma_start(
                    out=x_buf[:, slot * F + lo:slot * F + hi],
                    in_=x_t[LAST][:, lo:hi],
                ).then_inc(dma_p[i], 16)
            # output pieces, each issued as soon as its producer finishes
            sync.wait_ge(fin_sem, 1)
            sync.dma_start(
                out=out_ext.ap()[:, 0:10], in_=out_sb[:, 0:10]
            ).then_inc(dma_o, 16)
            sync.wait_ge(fin_sem, 2)
            sync.dma_start(
                out=out_ext.ap()[:, 10:MC], in_=out_sb[:, 10:MC]
            ).then_inc(dma_o, 16)
            sync.wait_ge(dma_o, 48)

        # ---- DMA program B (ACT / HWDGE): odd rounds; epilogue copy ----
        @block.scalar
        def _(scalar):
            for r in range(1, NT - 4, 2):
                if r >= BUFS:
                    scalar.wait_ge(pe_tile, r - BUFS + 1)
                slot = r % BUFS
                scalar.dma_start(
                    out=x_buf[:, slot * F:(slot + 1) * F], in_=x_t[r]
                ).then_inc(dma_x[slot], 16)
            scalar.wait_ge(pe_tile, NT)
            scalar.copy(out_sb[:, MC:ML], psum_small[:, :]).then_inc(cp_sem, 1)
            # wait for the copy's writes to land before the DMA reads them
            scalar.wait_ge(cp_sem, 1)
            scalar.dma_start(
                out=out_ext.ap()[:, MC:ML], in_=out_sb[:, MC:ML]
            ).then_inc(dma_o, 16)

        # ---- VectorE: w generation, j-reduction, epilogue subreduce ----
        @block.vector
        def _(vector):
            vector.wait_ge(dma_a, 16)
            for t in range(NT):
                wg = vector.scalar_tensor_tensor(
                    out=w_sb[:, t * G:(t + 1) * G],
                    in0=aux_sb[:, 0:G],
                    scalar=aux_sb[:, 2 * G + t:2 * G + t + 1],
                    in1=aux_sb[:, G:2 * G],
                    op0=mybir.AluOpType.is_equal,
                    op1=mybir.AluOpType.mult,
                )
            wg.then_inc(wg_sem, 1)

            for r in range(NT):
                slot = r % BUFS
                if r >= BUFS:
                    vector.wait_ge(pe_tile, r - BUFS + 1)
                if r == LAST:
                    # three 13/13/14-chan pieces, pipelined with landings
                    for i in range(3):
                        lo, hi = R15[i]
                        vector.wait_ge(dma_p[i], 16)
                        vector.tensor_reduce(
                            out=xs_buf[:, slot * MV + lo // HW:
                                       slot * MV + hi // HW],
                            in_=x_buf[:, slot * F + lo:slot * F + hi
                                      ].rearrange("p (m j) -> p m j", j=HW),
                            axis=mybir.AxisListType.X,
                            op=mybir.AluOpType.add,
                        ).then_inc(red_sem, 1)
                    continue
                if r == PEN:
                    vector.wait_ge(dma_q[0], 16)
                else:
                    vector.wait_ge(dma_x[slot], _cum(r))
                vector.tensor_reduce(
                    out=xs_buf[:, slot * MV:(slot + 1) * MV],
                    in_=x_buf[:, slot * F:slot * F + DOFF].rearrange(
                        "p (m j) -> p m j", j=HW
                    ),
                    axis=mybir.AxisListType.X,
                    op=mybir.AluOpType.add,
                ).then_inc(red_sem, 1)

            for (mlo, mhi), need in SUBRED:
                vector.wait_ge(pe_big, need)
                vector.tensor_reduce(
                    out=out_sb[:, mlo:mhi],
                    in_=psum_big[:, mlo * HW:mhi * HW].rearrange(
                        "p (m j) -> p m j", j=HW
                    ),
                    axis=mybir.AxisListType.X,
                    op=mybir.AluOpType.add,
                ).then_inc(fin_sem, 1)

        # ---- TensorE: segment-sum accumulation (fp32) ----
        @block.tensor
        def _(tensor):
            tensor.wait_ge(wg_sem, 1)
            for r in range(NT):
                if r == PEN:
                    tensor.wait_ge(dma_q[1], 16)
                elif r != LAST:
                    tensor.wait_ge(dma_x[r % BUFS], _cum(r))
                slot = r % BUFS

                def chunk_mm(ci):
                    lo, hi = CHUNKS[ci]
                    if r == LAST:
                        tensor.wait_ge(dma_p[3 + ci], 16)
                    tensor.matmul(
                        out=psum_big[:, lo:hi],
                        lhsT=wt,
                        rhs=x_buf[:, slot * F + DOFF + lo:slot * F + DOFF + hi],
                        start=(r == 0),
                        stop=(r == NT - 1),
                    ).then_inc(pe_big, 1)

                def small_mm():
                    tensor.wait_ge(red_sem, NT + 2 if r == LAST else r + 1)
                    tensor.matmul(
                        out=psum_small[:, :],
                        lhsT=wt,
                        rhs=xs_buf[:, slot * MV:(slot + 1) * MV],
                        start=(r == 0),
                        stop=(r == NT - 1),
                    ).then_inc(pe_tile, 1)

                wt = w_sb[:, r * G:(r + 1) * G]
                if r == LAST:
                    # small between the chunks: the psum_small -> copy ->
                    # out path starts ~1.5 us earlier than if it queued
                    # behind both chunk matmuls
                    chunk_mm(0)
                    small_mm()
                    chunk_mm(1)
                else:
                    chunk_mm(0)
                    chunk_mm(1)
                    small_mm()

    return nc


def _prepare(x, idx):
    x = np.asarray(x)
    if x.dtype != np.float32:
        x = x.astype(np.float32)
    idx = np.asarray(idx).astype(np.int64)
    counts = np.bincount(idx, minlength=G).astype(np.float64)
    scale = np.where(counts > 0, 1.0 / (counts * HW), 0.0).astype(np.float32)
    aux = np.zeros((P, G + G + NT), np.float32)
    aux[:, 0:G] = np.arange(G, dtype=np.float32)[None, :]
    aux[:, G:2 * G] = scale[None, :]
    aux[:, 2 * G:] = idx.reshape(NT, P).T.astype(np.float32)
    xr = x.reshape(N, M, HW)
    in_maps = []
    for k in range(CORES):
        # MV channels (MC..63) first, then PE channels (0..MC)
        sl = xr[:, k * ML:(k + 1) * ML, :]
        shard = np.concatenate([sl[:, MC:, :], sl[:, :MC, :]], axis=1)
        shard = np.ascontiguousarray(shard).reshape(N, F)
        in_maps.append({"x": shard, "aux": aux})
    return in_maps


def run(x, tensor_list_assignmentindices, trace=False):
    in_maps = _prepare(x, tensor_list_assignmentindices)
    nc = _build()
    res = run_bass_kernel_spmd(nc, in_maps, core_ids=list(range(CORES)), trace=trace)
    outs = [np.asarray(r["out"]) for r in res.results]
    out = np.concatenate(outs, axis=1)
    return out.reshape(G, M, 1, 1).astype(np.float32), res.exec_time_ns


def kernel(**inputs):
    out, _ = run(inputs["x"], inputs["tensor_list_assignmentindices"], trace=False)
    return out
